# revision 45
# baseline (speedup 1.0000x reference)
"""CBAM kernel for Trainium2, 8-core data-parallel (4 batches per core).

Layout trick: per core the shard is [12544, 256] (4 batches x 3136 spatial x 256ch).
Split into 2 batch-PAIRS of [6272, 256]. Within a pair, flat row r = 49*p + n
(p in [0,128), n in [0,49)) puts batch = p//64 exactly on a 64-partition group
(3136 = 64*49), giving fully contiguous per-partition DMA (50KB runs) and
letting every compute op span all 128 partitions.

bf16 design (196us baseline -> ~149us): x is cast fp32->bf16 during the
SWDGE in-DMA (full HBM rate); all full-data elementwise work runs on DVE
in bf16 2x_1P single-port modes, so SWDGE descriptor generation on GpSimd
is never starved by a DVE 2-port op.  Key measured facts driving the
structure: DVE TT/reduce never exceed 2x_1P (reduce is 1x, inherently
serial per lane -- hence the TT pre-folds); a single fp32 operand demotes
a DVE op to 1x; SWDGE bf16->fp32 cast-out runs at ~half line rate (hence
fp32 staging + HWDGE out); walrus cannot codegen TensorTensor on Pool.

  phase1 (overlaps DMA-in): channel-max via DVE bf16 tensor_max chain;
          channel-sum on PE as two-block bf16 matmuls with a
          block-diagonal ones mask, PSUM-accumulated in two halves.
  MLP:    tiny fp32/bf16 mix; b2 folded into an augmented w2 matmul
          ([HID+1] with a ones row); relu on ACT with b1 as bias AP; ONE
          sigmoid over the [2,2,C] PSUM tile; the avg+max sum happens in
          the gate-broadcast matmul via PSUM accumulation.  Pair 1's max
          chain interleaves into pair 0's phase2 DVE stream (2 chunks per
          iteration) and mlp(1) launches mid-phase2(0).
  phase2: per chunk: in-place bf16 TT gate multiply, then per stat a
          256->64 TT fold tree (2x mode) before the 1x tensor_reduce;
          smax/savg live in one [128,2,49] tile.
  conv:   stats round trip via ONE [ic][b][s] DRAM tensor: two full-width
          writes + a single combined reload (SWDGE, idle gpsimd queue);
          7x7 conv as 14 bf16 PE band matmuls; 1/C folded into the avg
          bands host-side.
  phase4: out = xg * sg per 256-block into rotating fp32 staging buffers
          (PH4_ENGINES splits blocks DVE/ACT per pair to match when each
          engine is free), then plain full-rate HWDGE out-DMAs on sync.
"""

import numpy as np
from contextlib import ExitStack

import concourse.bass as bass
import concourse.tile as tile
from concourse import mybir
from concourse._compat import with_exitstack
from concourse.tile import add_dep_helper

F32 = mybir.dt.float32
BF16 = mybir.dt.bfloat16

C = 256
HID = 16
NPAIR = 2          # batch pairs per core
NBLK = 49          # 256-ch blocks per pair free dim (3136 = 64*49)
CHUNK = 7          # blocks per DMA chunk
NCHUNK = NBLK // CHUNK
ROWS_PAIR = 128 * NBLK   # 6272
ROWS_CORE = NPAIR * ROWS_PAIR  # 12544
H = W = 56
SP = H * W  # 3136
N_CORES = 8
AVG_SCALE = 1.0 / C
# phase-4 engine per (pair, chunk): A=ACT, D=DVE.  Pair 0's first chunks go
# to ACT (DVE still busy with pair 1's phase2), the rest to DVE once it
# frees; pair 1 leans on DVE so ACT is open for conv 1's sigmoid.
PH4_ENGINES = ("AAADDDD", "DDDDDAA")

MU = mybir.AluOpType
AF = mybir.ActivationFunctionType


def _ap(handle_ap, offset_elems, dims):
    """Raw AP over a DRAM tensor: dims = [[step, count], ...] in elements."""
    base = handle_ap[tuple([slice(None)] * len(handle_ap.shape))]
    return bass.AP(tensor=base.tensor, offset=base.offset + offset_elems, ap=dims)


@with_exitstack
def _emit(ctx: ExitStack, tc: tile.TileContext):
    nc = tc.nc

    x_d = nc.dram_tensor("x", [ROWS_CORE, C], F32, kind="ExternalInput")
    w1h_d = nc.dram_tensor("w1h", [128, 2, HID], F32, kind="ExternalInput")
    w1sh_d = nc.dram_tensor("w1sh", [128, 2, HID], F32, kind="ExternalInput")
    w2aug_d = nc.dram_tensor("w2aug", [HID + 1, C], F32, kind="ExternalInput")
    b1c_d = nc.dram_tensor("b1c", [HID, 1], F32, kind="ExternalInput")
    bands_d = nc.dram_tensor("bands", [H, 14, H], F32, kind="ExternalInput")
    ident_d = nc.dram_tensor("ident", [128, 128], F32, kind="ExternalInput")
    mask2_d = nc.dram_tensor("mask2", [128, 2], F32, kind="ExternalInput")
    mask2t_d = nc.dram_tensor("mask2t", [2, 128], F32, kind="ExternalInput")
    convb_d = nc.dram_tensor("convb", [H, 1], F32, kind="ExternalInput")
    out_d = nc.dram_tensor("out", [ROWS_CORE, C], F32, kind="ExternalOutput")

    # DRAM scratch for the conv-input / spatial-gate reshuffles.  The two
    # conv input stats live in ONE [ic][b][s] tensor so the round trip is
    # two parallel writes plus a single combined reload.
    sstat_d = nc.dram_tensor("sstat_s", [NPAIR, 2, 2, SP], F32)
    sg_d = nc.dram_tensor("sg_s", [NPAIR, ROWS_PAIR], F32)

    xv = x_d[:, :].rearrange("(q p n) c -> q p n c", q=NPAIR, p=128)
    ov = out_d[:, :].rearrange("(q p n) c -> q p n c", q=NPAIR, p=128)

    constp = ctx.enter_context(tc.tile_pool(name="const", bufs=1))
    bigp = ctx.enter_context(tc.tile_pool(name="big", bufs=1))
    workp = ctx.enter_context(tc.tile_pool(name="work", bufs=1))
    psp1 = ctx.enter_context(tc.tile_pool(name="ps1", bufs=1, space="PSUM"))
    psp2 = ctx.enter_context(tc.tile_pool(name="ps2", bufs=2, space="PSUM"))

    # ---- constants to SBUF ----
    def const_load(name, shape, dram):
        t = constp.tile(shape, F32, tag=name)
        nc.sync.dma_start(t[tuple([slice(None)] * len(shape))], dram)
        return t

    w1h = const_load("w1h", [128, 2, HID], w1h_d[:, :, :])
    w1sh = const_load("w1sh", [128, 2, HID], w1sh_d[:, :, :])
    w2aug = const_load("w2aug", [HID + 1, C], w2aug_d[:, :])
    b1c = const_load("b1c", [HID, 1], b1c_d[:, :])
    bands = const_load("bands", [H, 14, H], bands_d[:, :, :])
    ident = const_load("ident", [128, 128], ident_d[:, :])
    mask2 = const_load("mask2", [128, 2], mask2_d[:, :])
    mask2t = const_load("mask2t", [2, 128], mask2t_d[:, :])
    convb = const_load("convb", [H, 1], convb_d[:, :])

    # ACT funnel copies: every fp32 matmul operand must reach PE with deps on
    # at most one engine (fused-LDWEIGHTS fp32 matmuls tolerate 1 sync wait).
    # ACT (not DVE) so no 2-port DVE op ever starves SWDGE descriptor gen.
    def funnel(name, src, shape, dtype=F32):
        t = constp.tile(shape, dtype, tag=name)
        nc.scalar.copy(t[tuple([slice(None)] * len(shape))],
                       src[tuple([slice(None)] * len(shape))])
        return t

    identb = funnel("identb", ident, [128, 128])
    identb16 = funnel("identb16", ident, [128, 128], BF16)
    w1hb16 = funnel("w1hb16", w1h, [128, 2, HID], BF16)
    w1shb = funnel("w1shb", w1sh, [128, 2, HID])
    w2augb = funnel("w2augb", w2aug, [HID + 1, C])
    bandsb16 = funnel("bandsb16", bands, [H, 14, H], BF16)
    mask2b = funnel("mask2b", mask2, [128, 2], BF16)
    mask2tb16 = funnel("mask2tb16", mask2t, [2, 128], BF16)

    # augmented-h tiles: ones row HID survives the relu write of rows 0..HID-1
    haugs = {}
    for q in range(NPAIR):
        haug_t = workp.tile([HID + 1, 2, 2], F32, tag=f"haug{q}")
        nc.gpsimd.memset(haug_t[:, :, :], 1.0)
        haugs[q] = haug_t

    prev = {}

    def phase1_dma(q):
        """Issue all in-DMAs (SWDGE cast fp32 HBM -> bf16 SBUF) up front so
        both pairs' loads stream back-to-back on the SDMA engines."""
        X = bigp.tile([128, NBLK, C], BF16, tag=f"x{q}")
        for k in range(NCHUNK):
            nc.gpsimd.dma_start(
                X[:, k * CHUNK : (k + 1) * CHUNK, :],
                xv[q, :, k * CHUNK : (k + 1) * CHUNK, :],
            )
        return X

    def phase1_start(q):
        aw = workp.tile([128, CHUNK, C], BF16, tag=f"aw{q}")
        chsum = psp2.tile([2, 2, C], F32, tag="chsum")
        return dict(aw=aw, chsum=chsum, first=None, last=None)

    def phase1_chunk(q, X, st, k):
        aw, chsum = st["aw"], st["chsum"]
        blk = X[:, k * CHUNK : (k + 1) * CHUNK, :]
        if k == 0:
            nc.vector.tensor_max(aw[:], blk, blk)
        else:
            nc.vector.tensor_max(aw[:], aw[:], blk)
        # channel sums on PE: 3 two-block matmuls + 1 single per chunk,
        # accumulating even blocks into half 0, odd into half 1 (the
        # trailing singles also land in half 0; the halves are added in
        # mlp()).  skip_group_check: the singles address a subregion of
        # the accumulation group.
        b0 = k * CHUNK
        for m in range(4):
            if m < 3:
                rhs = X[:, b0 + 2 * m : b0 + 2 * m + 2, :]
                out = chsum[:, :, :]
            else:
                rhs = X[:, b0 + 6, :]
                out = chsum[:, 0, :]
            mm = nc.tensor.matmul(
                out, lhsT=mask2b[:], rhs=rhs,
                start=(k == 0 and m == 0),
                stop=(k == NCHUNK - 1 and m == 3),
                skip_group_check=True,
            )
            if st["first"] is None:
                st["first"] = mm
            st["last"] = mm

    def phase1_finish(q, st):
        aw = st["aw"]
        if "last_chsum" in prev:
            add_dep_helper(st["first"].ins, prev["last_chsum"].ins, sync=False,
                           reason="pair order on PE")
        prev["last_chsum"] = st["last"]
        # folds
        nc.vector.tensor_max(aw[:, 0:3, :], aw[:, 0:3, :], aw[:, 3:6, :])
        nc.vector.tensor_max(aw[:, 0, :], aw[:, 0, :], aw[:, 1, :])
        nc.vector.tensor_max(aw[:, 0, :], aw[:, 0, :], aw[:, 2, :])
        nc.vector.tensor_max(aw[:, 0, :], aw[:, 0, :], aw[:, 6, :])
        return aw[:, 0, :], st["chsum"]

    def phase1_compute(q, X):
        st = phase1_start(q)
        for k in range(NCHUNK):
            phase1_chunk(q, X, st, k)
        return phase1_finish(q, st)

    def mlp(q, acc, chsum):
        # fold the two chsum accumulation halves; TT may read only one PSUM
        # input, so stage half 1 through SBUF on ACT first
        sum_h1 = workp.tile([2, C], F32, tag=f"sumh1{q}")
        nc.scalar.copy(sum_h1[:], chsum[:, 1, :])
        sum_sb = workp.tile([2, C], F32, tag=f"sum{q}")
        nc.vector.tensor_add(sum_sb[:], chsum[:, 0, :], sum_h1[:])
        # stats with c on partitions; max path stays bf16 end-to-end (PE
        # transpose of the bf16 max-acc -> bf16 PSUM -> bf16 reduce)
        statsM = workp.tile([128, 2, 2], BF16, tag=f"statsM{q}")
        statsA = workp.tile([128, 2, 2], F32, tag=f"statsA{q}")
        mlp_ps = psp1.tile([128, 16], F32, tag="mlp")
        for h2 in range(2):
            tp = psp1.tile([128, 128], BF16, tag="tp")
            nc.tensor.transpose(tp[:], acc[:, h2 * 128 : (h2 + 1) * 128],
                                identb16[:])
            nc.vector.tensor_reduce(
                out=statsM[:, h2, :],
                in_=tp[:].rearrange("c (b p) -> c b p", b=2),
                axis=mybir.AxisListType.X, op=MU.max,
            )
            nc.tensor.transpose(
                mlp_ps[:, 2 * h2 : 2 * h2 + 2],
                sum_sb[:, h2 * 128 : (h2 + 1) * 128],
                identb[0:2, 0:2],
            )
            nc.scalar.copy(statsA[:, h2, :], mlp_ps[:, 2 * h2 : 2 * h2 + 2])

        for stat in range(2):
            for h2 in range(2):
                w1x = w1shb[:, h2, :] if stat == 0 else w1hb16[:, h2, :]
                rhs = statsA[:, h2, :] if stat == 0 else statsM[:, h2, :]
                nc.tensor.matmul(
                    mlp_ps[0:HID, 4 + 2 * stat : 6 + 2 * stat],
                    lhsT=w1x, rhs=rhs,
                    start=(h2 == 0), stop=(h2 == 1),
                )
        # h = relu(h_ps + b1) on ACT, into the augmented-h tile (ones row
        # HID was memset at setup and survives)
        haug = haugs[q]
        nc.scalar.activation(
            out=haug[0:HID, :, :],
            in_=mlp_ps[0:HID, 4:8].rearrange("p (s b) -> p s b", s=2),
            func=AF.Relu, bias=b1c[:], scale=1.0,
        )
        # cg rows: sigmoid(w2_aug.T @ h_aug) per stat (one ACT op over the
        # [2, 2, C] PSUM tile), then the per-stat sum happens inside the
        # broadcast matmul via PSUM accumulation.
        cgps = psp1.tile([2, 2, C], F32, tag="cgps")
        nc.tensor.matmul(cgps[:, 0, :], lhsT=haug[:, 0, :], rhs=w2augb[:],
                         start=True, stop=True)
        nc.tensor.matmul(cgps[:, 1, :], lhsT=haug[:, 1, :], rhs=w2augb[:],
                         start=True, stop=True)
        sig4 = workp.tile([2, 2, C], BF16, tag=f"sig4{q}")
        nc.scalar.activation(out=sig4[:], in_=cgps[:, :, :], func=AF.Sigmoid,
                             bias=0.0, scale=1.0)
        cgb_ps = psp1.tile([128, C], F32, tag="cgb")
        nc.tensor.matmul(cgb_ps[:], lhsT=mask2tb16[:], rhs=sig4[:, 0, :],
                         start=True, stop=False)
        nc.tensor.matmul(cgb_ps[:], lhsT=mask2tb16[:], rhs=sig4[:, 1, :],
                         start=False, stop=True)
        cgb = workp.tile([128, C], BF16, tag=f"cgb{q}")
        nc.scalar.copy(cgb[:], cgb_ps[:])
        return cgb

    def phase2(q, X, cgb, interleave=None):
        # per chunk: one 2x TT multiply applies the channel gate in place;
        # then GpSimd folds xg 256->128 (max and add variants) while DVE
        # folds 128->64 and runs the (inherently 1x) final reduces on a
        # quarter of the data.  Double-buffered Z scratch decouples chunks.
        # ss[:, 0, :] = savg, ss[:, 1, :] = smax -- one tile, stat slots in
        # the conv's ic order, so the reshuffle is two partition-half writes
        # plus one straight positive-stride reload
        ss = workp.tile([128, 2, NBLK], F32, tag=f"ss{q}")
        savg = ss[:, 0, :]
        smax = ss[:, 1, :]
        zs = []
        for ab in "AB":
            zm_t = workp.tile([128, CHUNK, 128], BF16, tag=f"zm{q}{ab}")
            za_t = workp.tile([128, CHUNK, 128], BF16, tag=f"za{q}{ab}")
            zs.append((zm_t, za_t))
        cgb_rep = bass.AP(tensor=cgb.tensor, offset=cgb.offset,
                          ap=[cgb.ap[0], [0, CHUNK], cgb.ap[1]])
        for k in range(NCHUNK):
            blk = X[:, k * CHUNK : (k + 1) * CHUNK, :]
            ks = slice(k * CHUNK, (k + 1) * CHUNK)
            nc.vector.tensor_tensor(out=blk, in0=blk, in1=cgb_rep, op=MU.mult)
            Zm, Za = zs[k % 2]
            nc.vector.tensor_max(Zm[:], blk[:, :, 0:128], blk[:, :, 128:256])
            nc.vector.tensor_add(Za[:], blk[:, :, 0:128], blk[:, :, 128:256])
            nc.vector.tensor_max(Zm[:, :, 0:64], Zm[:, :, 0:64],
                                 Zm[:, :, 64:128])
            nc.vector.tensor_reduce(
                out=smax[:, ks], in_=Zm[:, :, 0:64],
                axis=mybir.AxisListType.X, op=MU.max,
            )
            nc.vector.tensor_add(Za[:, :, 0:64], Za[:, :, 0:64],
                                 Za[:, :, 64:128])
            nc.vector.tensor_reduce(
                out=savg[:, ks], in_=Za[:, :, 0:64],
                axis=mybir.AxisListType.X, op=MU.add,
            )
            # stream this chunk's stats to DRAM now so conv() only pays
            # the reload latency after the last chunk
            nc.gpsimd.dma_start(
                _ap(sstat_d, q * 4 * SP + k * CHUNK,
                    [[NBLK, 128], [2 * SP, 2], [1, CHUNK]]),
                ss[:, :, k * CHUNK : (k + 1) * CHUNK],
            )
            if interleave is not None:
                interleave(k)
        return ss

    def conv(q, ss):
        # conv reshuffles ride the gpsimd (SWDGE) queue: it is idle after
        # the in-DMAs, so the conv round trip never queues behind the
        # out-stream on sync.  DRAM addr = ic*2*SP + b*SP + s, affine per
        # 64-partition batch half, so each write is one DMA per half and
        # the reload is a single DMA covering both stats and batches.
        s_sb = workp.tile([H, 2, 2, 62], BF16, tag=f"ssb{q}")  # [h, ic, b, w+pad]
        nc.gpsimd.memset(s_sb[:], 0.0)
        nc.gpsimd.dma_start(
            s_sb[0:H, :, :, 3 : 3 + W],
            _ap(sstat_d, q * 4 * SP,
                [[W, H], [2 * SP, 2], [SP, 2], [1, W]]),
        )
        s_sb2 = workp.tile([H, 2, 2, 62], BF16, tag=f"ssb2{q}")
        nc.scalar.copy(s_sb2[:], s_sb[:])
        conv_ps = psp2.tile([H, 2, W], F32, tag="conv")
        for ic in range(2):
            for dw in range(7):
                j = ic * 7 + dw
                nc.tensor.matmul(
                    conv_ps[:], lhsT=bandsb16[:, j, :],
                    rhs=s_sb2[:, ic, :, dw : dw + W],
                    start=(j == 0), stop=(j == 13),
                )
        sg_hw = workp.tile([H, 2, W], F32, tag=f"sghw{q}")
        nc.scalar.activation(
            out=sg_hw[:], in_=conv_ps[:], func=AF.Sigmoid,
            bias=convb[:], scale=1.0,
        )
        nc.gpsimd.dma_start(
            _ap(sg_d, q * ROWS_PAIR, [[W, H], [SP, 2], [1, W]]), sg_hw[:]
        )
        sg32 = workp.tile([128, NBLK], F32, tag=f"sg32{q}")
        nc.gpsimd.dma_start(
            sg32[:], _ap(sg_d, q * ROWS_PAIR, [[NBLK, 128], [1, NBLK]])
        )
        return (sg32,)

    # fp32 out-staging: the gate multiply writes fp32 (engine casts are
    # free) so the out-DMA is a plain full-rate HWDGE transfer -- the SWDGE
    # bf16->fp32 cast DMA only sustains ~half line rate.
    stgs = []
    for b in range(NCHUNK):
        stg_t = bigp.tile([128, CHUNK, C], F32, tag=f"stg{b}")
        stgs.append(stg_t)

    def phase4(q, X, sg32):
        # chunk-granular engine split so each out-DMA waits on one engine.
        # Pair 0's multiply runs while DVE is busy with pair 1's phase2, so
        # it leans on ACT; pair 1's runs in the tail when DVE is free.
        for k in range(NCHUNK):
            stg = stgs[k]
            for n in range(k * CHUNK, (k + 1) * CHUNK):
                j = n - k * CHUNK
                if PH4_ENGINES[q][k] == "D":
                    nc.vector.tensor_scalar_mul(
                        stg[:, j, :], X[:, n, :], sg32[:, n : n + 1]
                    )
                else:
                    nc.scalar.mul(stg[:, j, :], X[:, n, :],
                                  mul=sg32[:, n : n + 1])
            nc.sync.dma_start(
                ov[q, :, k * CHUNK : (k + 1) * CHUNK, :], stg[:, :, :]
            )

    # pipeline-ordered emission: all in-DMAs first (they stream on SDMA
    # regardless of engine progress); pair 0's whole compute pipeline is
    # queued before pair 1's DVE work so pair 0's conv inputs are not stuck
    # behind pair 1's max chain in the DVE program order.
    X0 = phase1_dma(0)
    X1 = phase1_dma(1)
    acc0, chsum0 = phase1_compute(0, X0)
    cgb0 = mlp(0, acc0, chsum0)
    # pair 1's max-chain ops interleave into pair 0's phase2 stream on DVE
    # (two chunks per iteration), and mlp(1) launches mid-phase2(0) so cgb1
    # is ready the moment phase2(0) finishes
    st1 = phase1_start(1)
    mlp1_out = {}

    def ilv(k):
        if k <= 2:
            phase1_chunk(1, X1, st1, 2 * k)
            phase1_chunk(1, X1, st1, 2 * k + 1)
        elif k == 3:
            phase1_chunk(1, X1, st1, 6)
            acc1, chsum1 = phase1_finish(1, st1)
            mlp1_out["cgb"] = mlp(1, acc1, chsum1)

    ss0 = phase2(0, X0, cgb0, interleave=ilv)
    cgb1 = mlp1_out["cgb"]
    sg0 = conv(0, ss0)
    ss1 = phase2(1, X1, cgb1)
    sg1 = conv(1, ss1)
    phase4(0, X0, *sg0)
    phase4(1, X1, *sg1)


def _split_evsem_clears(nc):
    """This walrus build rejects EVENT_SEMAPHORE_RANGE_CLEAR over wide sem
    ranges ("ISA wrong length"); split into clears of <=3 sems."""
    for f in nc.m.functions:
        for blk in f.blocks:
            il = blk.instructions
            for i in range(len(il)):
                inst = il[i]
                if type(inst).__name__ != 'InstISA':
                    continue
                d = inst.ant_dict
                if d is None or 'range_first' not in d or 'range_last' not in d:
                    continue
                first, last = d['range_first'], d['range_last']
                if last - first + 1 <= 3:
                    continue
                si = inst.sync_info
                import copy
                reps = []
                a = first
                while a <= last:
                    b = min(a + 2, last)
                    cl = copy.deepcopy(inst)
                    cl.name = f"I-ws{nc.next_id()}"
                    cd = cl.ant_dict
                    cd['range_first'] = a
                    cd['range_last'] = b
                    reps.append(cl)
                    a = b + 1
                reps[0].sync_info = si
                il[i] = reps[0]
                for j, r in enumerate(reps[1:]):
                    il.insert(i + 1 + j, r)
                break


def _split_waits(nc):
    """Walrus in this toolchain accepts at most ONE sync wait per engine
    instruction; Tile freely emits several.  Split the surplus onto injected
    drain carriers (cloned from native Tile drains so they serialize
    correctly) placed immediately before the instruction -- same engine, so
    per-engine program order and semantics are unchanged."""
    import copy

    proto = {}
    for f in nc.m.functions:
        for blk in f.blocks:
            for inst in blk.instructions:
                if type(inst).__name__ == 'InstDrain' and inst.engine not in proto:
                    proto[inst.engine] = inst
    for f in nc.m.functions:
        for blk in f.blocks:
            il = blk.instructions
            i = 0
            while i < len(il):
                inst = il[i]
                si = inst.sync_info
                if si is None or len(si.on_wait) <= 1:
                    i += 1
                    continue
                waits = list(si.on_wait)
                eng = inst.engine
                for w in waits[:-1]:
                    nop = copy.deepcopy(proto[eng])
                    nop.name = f"I-ws{nc.next_id()}"
                    nop.sync_info = type(si)(on_wait=[w], on_update=[])
                    il.insert(i, nop)
                    i += 1
                inst.sync_info = type(si)(
                    on_wait=[waits[-1]], on_update=list(si.on_update)
                )
                i += 1


_NC = {}


def _get_nc(split=True):
    if split not in _NC:
        nc = bass.Bass()
        with tile.TileContext(nc) as tc:
            _emit(tc)
        if split:
            _split_waits(nc)
            _split_evsem_clears(nc)
        _NC[split] = nc
    return _NC[split]


def _host_inputs(w1, b1, w2, b2, conv_w, conv_b):
    w1 = np.asarray(w1, np.float32)
    w2 = np.asarray(w2, np.float32)
    b2 = np.asarray(b2, np.float32)
    w1h = np.ascontiguousarray(w1.reshape(2, 128, HID).transpose(1, 0, 2))
    w1sh = np.ascontiguousarray(w1h / float(SP))
    # augmented w2: [HID+1, C] with b2 as the last row (paired with a ones
    # row in h_aug, so the matmul adds the bias)
    w2aug = np.concatenate([w2.reshape(HID, C), b2.reshape(1, C)], axis=0)
    w2aug = np.ascontiguousarray(w2aug.astype(np.float32))
    b1c = np.ascontiguousarray(np.asarray(b1, np.float32).reshape(HID, 1))
    cw = np.asarray(conv_w, np.float32).reshape(7, 7, 2)
    bands = np.zeros((H, 14, H), np.float32)
    for ic in range(2):
        for dw in range(7):
            for dh in range(7):
                d = dh - 3  # hs - ho
                v = cw[dh, dw, ic]
                if d >= 0:
                    idx = np.arange(0, H - d)
                    bands[idx + d, ic * 7 + dw, idx] = v
                else:
                    idx = np.arange(-d, H)
                    bands[idx + d, ic * 7 + dw, idx] = v
    # savg is an UNSCALED channel sum on-device; fold the 1/C mean into the
    # avg-input (ic=0) bands
    bands[:, 0:7, :] *= AVG_SCALE
    ident = np.eye(128, dtype=np.float32)
    mask2 = np.zeros((128, 2), np.float32)
    mask2[0:64, 0] = 1.0
    mask2[64:128, 1] = 1.0
    mask2t = np.ascontiguousarray(mask2.T)
    convb = np.full((H, 1), np.asarray(conv_b, np.float32).reshape(-1)[0],
                    np.float32)
    return dict(w1h=w1h, w1sh=w1sh, w2aug=w2aug, b1c=b1c,
                bands=bands, ident=ident, mask2=mask2, mask2t=mask2t,
                convb=convb)


def kernel(x, w1, b1, w2, b2, conv_w, conv_b, _trace=False):
    from concourse.bass_utils import run_bass_kernel_spmd

    nc = _get_nc()
    consts = _host_inputs(w1, b1, w2, b2, conv_w, conv_b)
    xs = np.ascontiguousarray(np.asarray(x, np.float32)).reshape(8, ROWS_CORE, C)
    in_maps = [dict(consts, x=xs[i]) for i in range(N_CORES)]
    res = run_bass_kernel_spmd(nc, in_maps, core_ids=list(range(N_CORES)),
                               trace=_trace)
    out = np.stack([r["out"] for r in res.results])  # [8, 12544, 256]
    out = out.reshape(32, H, W, C)
    if _trace:
        kernel.last_results = res
    return out


# revision 46
# speedup vs baseline: 1.1242x; 1.1242x over previous
"""CBAM kernel for Trainium2, 8-core data-parallel (4 batches per core).

Layout trick: per core the shard is [12544, 256] (4 batches x 3136 spatial x 256ch).
Split into 2 batch-PAIRS of [6272, 256]. Within a pair, flat row r = 49*p + n
(p in [0,128), n in [0,49)) puts batch = p//64 exactly on a 64-partition group
(3136 = 64*49), giving fully contiguous per-partition DMA (50KB runs) and
letting every compute op span all 128 partitions.

bf16 design (196us baseline -> ~149us): x is cast fp32->bf16 during the
SWDGE in-DMA (full HBM rate); all full-data elementwise work runs on DVE
in bf16 2x_1P single-port modes, so SWDGE descriptor generation on GpSimd
is never starved by a DVE 2-port op.  Key measured facts driving the
structure: DVE TT/reduce never exceed 2x_1P (reduce is 1x, inherently
serial per lane -- hence the TT pre-folds); a single fp32 operand demotes
a DVE op to 1x; SWDGE bf16->fp32 cast-out runs at ~half line rate (hence
fp32 staging + HWDGE out); walrus cannot codegen TensorTensor on Pool.

  phase1 (overlaps DMA-in): channel-max via DVE bf16 tensor_max chain;
          channel-sum on PE as two-block bf16 matmuls with a
          block-diagonal ones mask, PSUM-accumulated in two halves.
  MLP:    tiny fp32/bf16 mix; b2 folded into an augmented w2 matmul
          ([HID+1] with a ones row); relu on ACT with b1 as bias AP; ONE
          sigmoid over the [2,2,C] PSUM tile; the avg+max sum happens in
          the gate-broadcast matmul via PSUM accumulation.  Pair 1's max
          chain interleaves into pair 0's phase2 DVE stream (2 chunks per
          iteration) and mlp(1) launches mid-phase2(0).
  phase2: per chunk: in-place bf16 TT gate multiply, then per stat a
          256->64 TT fold tree (2x mode) before the 1x tensor_reduce;
          smax/savg live in one [128,2,49] tile.
  conv:   stats round trip via ONE [ic][b][s] DRAM tensor: two full-width
          writes + a single combined reload (SWDGE, idle gpsimd queue);
          7x7 conv as 14 bf16 PE band matmuls; 1/C folded into the avg
          bands host-side.
  phase4: out = xg * sg per 256-block into rotating fp32 staging buffers
          (PH4_ENGINES splits blocks DVE/ACT per pair to match when each
          engine is free), then plain full-rate HWDGE out-DMAs on sync.
"""

import numpy as np
from contextlib import ExitStack

import concourse.bass as bass
import concourse.tile as tile
from concourse import mybir
from concourse._compat import with_exitstack
from concourse.tile import add_dep_helper

F32 = mybir.dt.float32
BF16 = mybir.dt.bfloat16

C = 256
HID = 16
NPAIR = 2          # batch pairs per core
NBLK = 49          # 256-ch blocks per pair free dim (3136 = 64*49)
CHUNK = 7          # blocks per DMA chunk
NCHUNK = NBLK // CHUNK
ROWS_PAIR = 128 * NBLK   # 6272
ROWS_CORE = NPAIR * ROWS_PAIR  # 12544
H = W = 56
SP = H * W  # 3136
N_CORES = 8
AVG_SCALE = 1.0 / C
# phase-4 engine per (pair, chunk): A=ACT, D=DVE.  Pair 0's first chunks go
# to ACT (DVE still busy with pair 1's phase2), the rest to DVE once it
# frees; pair 1 leans on DVE so ACT is open for conv 1's sigmoid.
PH4_ENGINES = ("AAADDDD", "DDDDDAA")

MU = mybir.AluOpType
AF = mybir.ActivationFunctionType


def _ap(handle_ap, offset_elems, dims):
    """Raw AP over a DRAM tensor: dims = [[step, count], ...] in elements."""
    base = handle_ap[tuple([slice(None)] * len(handle_ap.shape))]
    return bass.AP(tensor=base.tensor, offset=base.offset + offset_elems, ap=dims)


@with_exitstack
def _emit(ctx: ExitStack, tc: tile.TileContext):
    nc = tc.nc

    x_d = nc.dram_tensor("x", [ROWS_CORE, C], F32, kind="ExternalInput")
    w1h_d = nc.dram_tensor("w1h", [128, 2, HID], F32, kind="ExternalInput")
    w1sh_d = nc.dram_tensor("w1sh", [128, 2, HID], F32, kind="ExternalInput")
    w2aug_d = nc.dram_tensor("w2aug", [HID + 1, C], F32, kind="ExternalInput")
    b1c_d = nc.dram_tensor("b1c", [HID, 1], F32, kind="ExternalInput")
    bands_d = nc.dram_tensor("bands", [H, 14, H], F32, kind="ExternalInput")
    ident_d = nc.dram_tensor("ident", [128, 128], F32, kind="ExternalInput")
    mask2_d = nc.dram_tensor("mask2", [128, 2], F32, kind="ExternalInput")
    mask2t_d = nc.dram_tensor("mask2t", [2, 128], F32, kind="ExternalInput")
    convb_d = nc.dram_tensor("convb", [H, 1], F32, kind="ExternalInput")
    out_d = nc.dram_tensor("out", [ROWS_CORE, C], F32, kind="ExternalOutput")

    # DRAM scratch for the conv-input / spatial-gate reshuffles.  The two
    # conv input stats live in ONE [ic][b][s] tensor so the round trip is
    # two parallel writes plus a single combined reload.
    sstat_d = nc.dram_tensor("sstat_s", [NPAIR, 2, 2, SP], F32)
    sg_d = nc.dram_tensor("sg_s", [NPAIR, ROWS_PAIR], F32)

    xv = x_d[:, :].rearrange("(q p n) c -> q p n c", q=NPAIR, p=128)
    ov = out_d[:, :].rearrange("(q p n) c -> q p n c", q=NPAIR, p=128)

    constp = ctx.enter_context(tc.tile_pool(name="const", bufs=1))
    bigp = ctx.enter_context(tc.tile_pool(name="big", bufs=1))
    workp = ctx.enter_context(tc.tile_pool(name="work", bufs=1))
    psp1 = ctx.enter_context(tc.tile_pool(name="ps1", bufs=1, space="PSUM"))
    psp2 = ctx.enter_context(tc.tile_pool(name="ps2", bufs=2, space="PSUM"))

    # ---- constants to SBUF ----
    def const_load(name, shape, dram):
        t = constp.tile(shape, F32, tag=name)
        nc.sync.dma_start(t[tuple([slice(None)] * len(shape))], dram)
        return t

    w1h = const_load("w1h", [128, 2, HID], w1h_d[:, :, :])
    w1sh = const_load("w1sh", [128, 2, HID], w1sh_d[:, :, :])
    w2aug = const_load("w2aug", [HID + 1, C], w2aug_d[:, :])
    b1c = const_load("b1c", [HID, 1], b1c_d[:, :])
    bands = const_load("bands", [H, 14, H], bands_d[:, :, :])
    ident = const_load("ident", [128, 128], ident_d[:, :])
    mask2 = const_load("mask2", [128, 2], mask2_d[:, :])
    mask2t = const_load("mask2t", [2, 128], mask2t_d[:, :])
    convb = const_load("convb", [H, 1], convb_d[:, :])

    # ACT funnel copies: every fp32 matmul operand must reach PE with deps on
    # at most one engine (fused-LDWEIGHTS fp32 matmuls tolerate 1 sync wait).
    # ACT (not DVE) so no 2-port DVE op ever starves SWDGE descriptor gen.
    def funnel(name, src, shape, dtype=F32):
        t = constp.tile(shape, dtype, tag=name)
        nc.scalar.copy(t[tuple([slice(None)] * len(shape))],
                       src[tuple([slice(None)] * len(shape))])
        return t

    identb = funnel("identb", ident, [128, 128])
    identb16 = funnel("identb16", ident, [128, 128], BF16)
    w1hb16 = funnel("w1hb16", w1h, [128, 2, HID], BF16)
    w1shb = funnel("w1shb", w1sh, [128, 2, HID])
    w2augb = funnel("w2augb", w2aug, [HID + 1, C])
    bandsb16 = funnel("bandsb16", bands, [H, 14, H], BF16)
    mask2b = funnel("mask2b", mask2, [128, 2], BF16)
    mask2tb16 = funnel("mask2tb16", mask2t, [2, 128], BF16)

    # augmented-h tiles: ones row HID survives the relu write of rows 0..HID-1
    haugs = {}
    for q in range(NPAIR):
        haug_t = workp.tile([HID + 1, 2, 2], F32, tag=f"haug{q}")
        nc.gpsimd.memset(haug_t[:, :, :], 1.0)
        haugs[q] = haug_t

    prev = {}

    def phase1_dma(q):
        """Issue all in-DMAs (SWDGE cast fp32 HBM -> bf16 SBUF) up front so
        both pairs' loads stream back-to-back on the SDMA engines."""
        X = bigp.tile([128, NBLK, C], BF16, tag=f"x{q}")
        for k in range(NCHUNK):
            nc.gpsimd.dma_start(
                X[:, k * CHUNK : (k + 1) * CHUNK, :],
                xv[q, :, k * CHUNK : (k + 1) * CHUNK, :],
            )
        return X

    def phase1_start(q):
        aw = workp.tile([128, CHUNK, C], BF16, tag=f"aw{q}")
        chsum = psp2.tile([2, 2, C], F32, tag="chsum")
        return dict(aw=aw, chsum=chsum, first=None, last=None)

    def phase1_chunk(q, X, st, k):
        aw, chsum = st["aw"], st["chsum"]
        blk = X[:, k * CHUNK : (k + 1) * CHUNK, :]
        if k == 0:
            nc.vector.tensor_max(aw[:], blk, blk)
        else:
            nc.vector.tensor_max(aw[:], aw[:], blk)
        # channel sums on PE: 3 two-block matmuls + 1 single per chunk,
        # accumulating even blocks into half 0, odd into half 1 (the
        # trailing singles also land in half 0; the halves are added in
        # mlp()).  skip_group_check: the singles address a subregion of
        # the accumulation group.
        b0 = k * CHUNK
        for m in range(4):
            if m < 3:
                rhs = X[:, b0 + 2 * m : b0 + 2 * m + 2, :]
                out = chsum[:, :, :]
            else:
                rhs = X[:, b0 + 6, :]
                out = chsum[:, 0, :]
            mm = nc.tensor.matmul(
                out, lhsT=mask2b[:], rhs=rhs,
                start=(k == 0 and m == 0),
                stop=(k == NCHUNK - 1 and m == 3),
                skip_group_check=True,
            )
            if st["first"] is None:
                st["first"] = mm
            st["last"] = mm

    def phase1_finish(q, st):
        aw = st["aw"]
        if "last_chsum" in prev:
            add_dep_helper(st["first"].ins, prev["last_chsum"].ins, sync=False,
                           reason="pair order on PE")
        prev["last_chsum"] = st["last"]
        # folds
        nc.vector.tensor_max(aw[:, 0:3, :], aw[:, 0:3, :], aw[:, 3:6, :])
        nc.vector.tensor_max(aw[:, 0, :], aw[:, 0, :], aw[:, 1, :])
        nc.vector.tensor_max(aw[:, 0, :], aw[:, 0, :], aw[:, 2, :])
        nc.vector.tensor_max(aw[:, 0, :], aw[:, 0, :], aw[:, 6, :])
        return aw[:, 0, :], st["chsum"]

    def phase1_compute(q, X):
        st = phase1_start(q)
        for k in range(NCHUNK):
            phase1_chunk(q, X, st, k)
        return phase1_finish(q, st)

    def mlp(q, acc, chsum):
        # fold the two chsum accumulation halves; TT may read only one PSUM
        # input, so stage half 1 through SBUF on ACT first
        sum_h1 = workp.tile([2, C], F32, tag=f"sumh1{q}")
        nc.scalar.copy(sum_h1[:], chsum[:, 1, :])
        sum_sb = workp.tile([2, C], F32, tag=f"sum{q}")
        nc.vector.tensor_add(sum_sb[:], chsum[:, 0, :], sum_h1[:])
        # stats with c on partitions; max path stays bf16 end-to-end (PE
        # transpose of the bf16 max-acc -> bf16 PSUM -> bf16 reduce)
        statsM = workp.tile([128, 2, 2], BF16, tag=f"statsM{q}")
        statsA = workp.tile([128, 2, 2], F32, tag=f"statsA{q}")
        mlp_ps = psp1.tile([128, 16], F32, tag="mlp")
        for h2 in range(2):
            tp = psp1.tile([128, 128], BF16, tag="tp")
            nc.tensor.transpose(tp[:], acc[:, h2 * 128 : (h2 + 1) * 128],
                                identb16[:])
            nc.vector.tensor_reduce(
                out=statsM[:, h2, :],
                in_=tp[:].rearrange("c (b p) -> c b p", b=2),
                axis=mybir.AxisListType.X, op=MU.max,
            )
            nc.tensor.transpose(
                mlp_ps[:, 2 * h2 : 2 * h2 + 2],
                sum_sb[:, h2 * 128 : (h2 + 1) * 128],
                identb[0:2, 0:2],
            )
            nc.scalar.copy(statsA[:, h2, :], mlp_ps[:, 2 * h2 : 2 * h2 + 2])

        for stat in range(2):
            for h2 in range(2):
                w1x = w1shb[:, h2, :] if stat == 0 else w1hb16[:, h2, :]
                rhs = statsA[:, h2, :] if stat == 0 else statsM[:, h2, :]
                nc.tensor.matmul(
                    mlp_ps[0:HID, 4 + 2 * stat : 6 + 2 * stat],
                    lhsT=w1x, rhs=rhs,
                    start=(h2 == 0), stop=(h2 == 1),
                )
        # h = relu(h_ps + b1) on ACT, into the augmented-h tile (ones row
        # HID was memset at setup and survives)
        haug = haugs[q]
        nc.scalar.activation(
            out=haug[0:HID, :, :],
            in_=mlp_ps[0:HID, 4:8].rearrange("p (s b) -> p s b", s=2),
            func=AF.Relu, bias=b1c[:], scale=1.0,
        )
        # cg rows: sigmoid(w2_aug.T @ h_aug) per stat (one ACT op over the
        # [2, 2, C] PSUM tile), then the per-stat sum happens inside the
        # broadcast matmul via PSUM accumulation.
        cgps = psp1.tile([2, 2, C], F32, tag="cgps")
        nc.tensor.matmul(cgps[:, 0, :], lhsT=haug[:, 0, :], rhs=w2augb[:],
                         start=True, stop=True)
        nc.tensor.matmul(cgps[:, 1, :], lhsT=haug[:, 1, :], rhs=w2augb[:],
                         start=True, stop=True)
        sig4 = workp.tile([2, 2, C], BF16, tag=f"sig4{q}")
        nc.scalar.activation(out=sig4[:], in_=cgps[:, :, :], func=AF.Sigmoid,
                             bias=0.0, scale=1.0)
        cgb_ps = psp1.tile([128, C], F32, tag="cgb")
        nc.tensor.matmul(cgb_ps[:], lhsT=mask2tb16[:], rhs=sig4[:, 0, :],
                         start=True, stop=False)
        nc.tensor.matmul(cgb_ps[:], lhsT=mask2tb16[:], rhs=sig4[:, 1, :],
                         start=False, stop=True)
        cgb = workp.tile([128, C], BF16, tag=f"cgb{q}")
        nc.scalar.copy(cgb[:], cgb_ps[:])
        return cgb

    def phase2(q, X, cgb, interleave=None):
        # per chunk: one 2x TT multiply applies the channel gate in place;
        # then GpSimd folds xg 256->128 (max and add variants) while DVE
        # folds 128->64 and runs the (inherently 1x) final reduces on a
        # quarter of the data.  Double-buffered Z scratch decouples chunks.
        # ss[:, 0, :] = savg, ss[:, 1, :] = smax -- one tile, stat slots in
        # the conv's ic order, so the reshuffle is two partition-half writes
        # plus one straight positive-stride reload
        ss = workp.tile([128, 2, NBLK], F32, tag=f"ss{q}")
        savg = ss[:, 0, :]
        smax = ss[:, 1, :]
        zs = []
        for ab in "AB":
            zm_t = workp.tile([128, CHUNK, 128], BF16, tag=f"zm{q}{ab}")
            za_t = workp.tile([128, CHUNK, 128], BF16, tag=f"za{q}{ab}")
            zs.append((zm_t, za_t))
        cgb_rep = bass.AP(tensor=cgb.tensor, offset=cgb.offset,
                          ap=[cgb.ap[0], [0, CHUNK], cgb.ap[1]])
        for k in range(NCHUNK):
            blk = X[:, k * CHUNK : (k + 1) * CHUNK, :]
            ks = slice(k * CHUNK, (k + 1) * CHUNK)
            nc.vector.tensor_tensor(out=blk, in0=blk, in1=cgb_rep, op=MU.mult)
            Zm, Za = zs[k % 2]
            nc.vector.tensor_max(Zm[:], blk[:, :, 0:128], blk[:, :, 128:256])
            nc.vector.tensor_add(Za[:], blk[:, :, 0:128], blk[:, :, 128:256])
            nc.vector.tensor_max(Zm[:, :, 0:64], Zm[:, :, 0:64],
                                 Zm[:, :, 64:128])
            nc.vector.tensor_reduce(
                out=smax[:, ks], in_=Zm[:, :, 0:64],
                axis=mybir.AxisListType.X, op=MU.max,
            )
            nc.vector.tensor_add(Za[:, :, 0:64], Za[:, :, 0:64],
                                 Za[:, :, 64:128])
            nc.vector.tensor_reduce(
                out=savg[:, ks], in_=Za[:, :, 0:64],
                axis=mybir.AxisListType.X, op=MU.add,
            )
            if interleave is not None:
                interleave(k)
        return ss

    def conv(q, ss):
        # conv reshuffles ride the gpsimd (SWDGE) queue: it is idle after
        # the in-DMAs, so the conv round trip never queues behind the
        # out-stream on sync.  DRAM addr = ic*2*SP + b*SP + s, affine per
        # 64-partition batch half, so each write is one DMA per half and
        # the reload is a single DMA covering both stats and batches.
        # two full-width writes (a 64-partition DMA only engages half the
        # SDMA engines); the flat p*49+n layout equals [b][s] order
        nc.gpsimd.dma_start(
            _ap(sstat_d, q * 4 * SP, [[NBLK, 128], [1, NBLK]]),
            ss[:, 0, :],
        )
        nc.gpsimd.dma_start(
            _ap(sstat_d, q * 4 * SP + 2 * SP, [[NBLK, 128], [1, NBLK]]),
            ss[:, 1, :],
        )
        s_sb = workp.tile([H, 2, 2, 62], BF16, tag=f"ssb{q}")  # [h, ic, b, w+pad]
        nc.gpsimd.memset(s_sb[:], 0.0)
        nc.gpsimd.dma_start(
            s_sb[0:H, :, :, 3 : 3 + W],
            _ap(sstat_d, q * 4 * SP,
                [[W, H], [2 * SP, 2], [SP, 2], [1, W]]),
        )
        s_sb2 = workp.tile([H, 2, 2, 62], BF16, tag=f"ssb2{q}")
        nc.scalar.copy(s_sb2[:], s_sb[:])
        conv_ps = psp2.tile([H, 2, W], F32, tag="conv")
        for ic in range(2):
            for dw in range(7):
                j = ic * 7 + dw
                nc.tensor.matmul(
                    conv_ps[:], lhsT=bandsb16[:, j, :],
                    rhs=s_sb2[:, ic, :, dw : dw + W],
                    start=(j == 0), stop=(j == 13),
                )
        sg_hw = workp.tile([H, 2, W], F32, tag=f"sghw{q}")
        nc.scalar.activation(
            out=sg_hw[:], in_=conv_ps[:], func=AF.Sigmoid,
            bias=convb[:], scale=1.0,
        )
        nc.gpsimd.dma_start(
            _ap(sg_d, q * ROWS_PAIR, [[W, H], [SP, 2], [1, W]]), sg_hw[:]
        )
        sg32 = workp.tile([128, NBLK], F32, tag=f"sg32{q}")
        nc.gpsimd.dma_start(
            sg32[:], _ap(sg_d, q * ROWS_PAIR, [[NBLK, 128], [1, NBLK]])
        )
        return (sg32,)

    # fp32 out-staging: the gate multiply writes fp32 (engine casts are
    # free) so the out-DMA is a plain full-rate HWDGE transfer -- the SWDGE
    # bf16->fp32 cast DMA only sustains ~half line rate.
    stgs = []
    for b in range(NCHUNK):
        stg_t = bigp.tile([128, CHUNK, C], F32, tag=f"stg{b}")
        stgs.append(stg_t)

    def phase4(q, X, sg32):
        # chunk-granular engine split so each out-DMA waits on one engine.
        # Pair 0's multiply runs while DVE is busy with pair 1's phase2, so
        # it leans on ACT; pair 1's runs in the tail when DVE is free.
        for k in range(NCHUNK):
            stg = stgs[k]
            for n in range(k * CHUNK, (k + 1) * CHUNK):
                j = n - k * CHUNK
                if PH4_ENGINES[q][k] == "D":
                    nc.vector.tensor_scalar_mul(
                        stg[:, j, :], X[:, n, :], sg32[:, n : n + 1]
                    )
                else:
                    nc.scalar.mul(stg[:, j, :], X[:, n, :],
                                  mul=sg32[:, n : n + 1])
            nc.sync.dma_start(
                ov[q, :, k * CHUNK : (k + 1) * CHUNK, :], stg[:, :, :]
            )

    # pipeline-ordered emission: all in-DMAs first (they stream on SDMA
    # regardless of engine progress); pair 0's whole compute pipeline is
    # queued before pair 1's DVE work so pair 0's conv inputs are not stuck
    # behind pair 1's max chain in the DVE program order.
    X0 = phase1_dma(0)
    X1 = phase1_dma(1)
    acc0, chsum0 = phase1_compute(0, X0)
    cgb0 = mlp(0, acc0, chsum0)
    # pair 1's max-chain ops interleave into pair 0's phase2 stream on DVE
    # (two chunks per iteration), and mlp(1) launches mid-phase2(0) so cgb1
    # is ready the moment phase2(0) finishes
    st1 = phase1_start(1)
    mlp1_out = {}

    def ilv(k):
        if k <= 2:
            phase1_chunk(1, X1, st1, 2 * k)
            phase1_chunk(1, X1, st1, 2 * k + 1)
        elif k == 3:
            phase1_chunk(1, X1, st1, 6)
            acc1, chsum1 = phase1_finish(1, st1)
            mlp1_out["cgb"] = mlp(1, acc1, chsum1)

    ss0 = phase2(0, X0, cgb0, interleave=ilv)
    cgb1 = mlp1_out["cgb"]
    sg0 = conv(0, ss0)
    ss1 = phase2(1, X1, cgb1)
    sg1 = conv(1, ss1)
    phase4(0, X0, *sg0)
    phase4(1, X1, *sg1)


def _split_evsem_clears(nc):
    """This walrus build rejects EVENT_SEMAPHORE_RANGE_CLEAR over wide sem
    ranges ("ISA wrong length"); split into clears of <=3 sems."""
    for f in nc.m.functions:
        for blk in f.blocks:
            il = blk.instructions
            for i in range(len(il)):
                inst = il[i]
                if type(inst).__name__ != 'InstISA':
                    continue
                d = inst.ant_dict
                if d is None or 'range_first' not in d or 'range_last' not in d:
                    continue
                first, last = d['range_first'], d['range_last']
                if last - first + 1 <= 3:
                    continue
                si = inst.sync_info
                import copy
                reps = []
                a = first
                while a <= last:
                    b = min(a + 2, last)
                    cl = copy.deepcopy(inst)
                    cl.name = f"I-ws{nc.next_id()}"
                    cd = cl.ant_dict
                    cd['range_first'] = a
                    cd['range_last'] = b
                    reps.append(cl)
                    a = b + 1
                reps[0].sync_info = si
                il[i] = reps[0]
                for j, r in enumerate(reps[1:]):
                    il.insert(i + 1 + j, r)
                break


def _split_waits(nc):
    """Walrus in this toolchain accepts at most ONE sync wait per engine
    instruction; Tile freely emits several.  Split the surplus onto injected
    drain carriers (cloned from native Tile drains so they serialize
    correctly) placed immediately before the instruction -- same engine, so
    per-engine program order and semantics are unchanged."""
    import copy

    proto = {}
    for f in nc.m.functions:
        for blk in f.blocks:
            for inst in blk.instructions:
                if type(inst).__name__ == 'InstDrain' and inst.engine not in proto:
                    proto[inst.engine] = inst
    for f in nc.m.functions:
        for blk in f.blocks:
            il = blk.instructions
            i = 0
            while i < len(il):
                inst = il[i]
                si = inst.sync_info
                if si is None or len(si.on_wait) <= 1:
                    i += 1
                    continue
                waits = list(si.on_wait)
                eng = inst.engine
                for w in waits[:-1]:
                    nop = copy.deepcopy(proto[eng])
                    nop.name = f"I-ws{nc.next_id()}"
                    nop.sync_info = type(si)(on_wait=[w], on_update=[])
                    il.insert(i, nop)
                    i += 1
                inst.sync_info = type(si)(
                    on_wait=[waits[-1]], on_update=list(si.on_update)
                )
                i += 1


_NC = {}


def _get_nc(split=True):
    if split not in _NC:
        nc = bass.Bass()
        with tile.TileContext(nc) as tc:
            _emit(tc)
        if split:
            _split_waits(nc)
            _split_evsem_clears(nc)
        _NC[split] = nc
    return _NC[split]


def _host_inputs(w1, b1, w2, b2, conv_w, conv_b):
    w1 = np.asarray(w1, np.float32)
    w2 = np.asarray(w2, np.float32)
    b2 = np.asarray(b2, np.float32)
    w1h = np.ascontiguousarray(w1.reshape(2, 128, HID).transpose(1, 0, 2))
    w1sh = np.ascontiguousarray(w1h / float(SP))
    # augmented w2: [HID+1, C] with b2 as the last row (paired with a ones
    # row in h_aug, so the matmul adds the bias)
    w2aug = np.concatenate([w2.reshape(HID, C), b2.reshape(1, C)], axis=0)
    w2aug = np.ascontiguousarray(w2aug.astype(np.float32))
    b1c = np.ascontiguousarray(np.asarray(b1, np.float32).reshape(HID, 1))
    cw = np.asarray(conv_w, np.float32).reshape(7, 7, 2)
    bands = np.zeros((H, 14, H), np.float32)
    for ic in range(2):
        for dw in range(7):
            for dh in range(7):
                d = dh - 3  # hs - ho
                v = cw[dh, dw, ic]
                if d >= 0:
                    idx = np.arange(0, H - d)
                    bands[idx + d, ic * 7 + dw, idx] = v
                else:
                    idx = np.arange(-d, H)
                    bands[idx + d, ic * 7 + dw, idx] = v
    # savg is an UNSCALED channel sum on-device; fold the 1/C mean into the
    # avg-input (ic=0) bands
    bands[:, 0:7, :] *= AVG_SCALE
    ident = np.eye(128, dtype=np.float32)
    mask2 = np.zeros((128, 2), np.float32)
    mask2[0:64, 0] = 1.0
    mask2[64:128, 1] = 1.0
    mask2t = np.ascontiguousarray(mask2.T)
    convb = np.full((H, 1), np.asarray(conv_b, np.float32).reshape(-1)[0],
                    np.float32)
    return dict(w1h=w1h, w1sh=w1sh, w2aug=w2aug, b1c=b1c,
                bands=bands, ident=ident, mask2=mask2, mask2t=mask2t,
                convb=convb)


def kernel(x, w1, b1, w2, b2, conv_w, conv_b, _trace=False):
    from concourse.bass_utils import run_bass_kernel_spmd

    nc = _get_nc()
    consts = _host_inputs(w1, b1, w2, b2, conv_w, conv_b)
    xs = np.ascontiguousarray(np.asarray(x, np.float32)).reshape(8, ROWS_CORE, C)
    in_maps = [dict(consts, x=xs[i]) for i in range(N_CORES)]
    res = run_bass_kernel_spmd(nc, in_maps, core_ids=list(range(N_CORES)),
                               trace=_trace)
    out = np.stack([r["out"] for r in res.results])  # [8, 12544, 256]
    out = out.reshape(32, H, W, C)
    if _trace:
        kernel.last_results = res
    return out
